# revision 1
# baseline (speedup 1.0000x reference)
"""MiniMind GQA attention block on 8 trn2 NeuronCores.

Sharding (per the TP-by-head hint): core c = (d, g) with d = c // 4 the
batch index (data parallel) and g = c % 4 the KV group (tensor parallel
over heads).  Each core computes q/k/v projections for its 4 query heads
and 1 KV head, RoPE, causal attention, and a partial output projection
through its slice of Wo rows; a grouped ReduceScatter (groups
[0-3], [4-7]) sums the partials and leaves each core with a distinct
128-row shard per 512-row sequence chunk.  The host only slices inputs
and concatenates output shards.

Everything on-chip runs transposed (feature dims on partitions) so the
softmax denominator folds into the PV matmul via a v|ones stationary
operand and no probability transpose is ever needed.
"""

import numpy as np
from contextlib import ExitStack

B, S, H = 2, 2048, 1024
NH, NKV, HD = 16, 4, 64
P = 128
NT = S // P            # 16 seq tiles
NCH = 4                # 512-wide sequence chunks
CHW = S // NCH         # 512
NCORES = 8

_prog_cache = {}


def _build():
    import concourse.bacc as bacc
    import concourse.mybir as mybir
    from concourse import tile

    F32 = mybir.dt.float32
    F32R = mybir.dt.float32r
    EXP = mybir.ActivationFunctionType.Exp
    MUL = mybir.AluOpType.mult
    ADD = mybir.AluOpType.add

    nc = bacc.Bacc()

    xT = nc.declare_dram_parameter("xT", [H, S], F32R, isOutput=False)
    wq = nc.declare_dram_parameter("wq", [H, 256], F32R, isOutput=False)
    wkv = nc.declare_dram_parameter("wkv", [H, 128], F32R, isOutput=False)
    wo = nc.declare_dram_parameter("wo", [256, H], F32R, isOutput=False)
    ct2 = nc.declare_dram_parameter("ct2", [128, S], F32, isOutput=False)
    st2 = nc.declare_dram_parameter("st2", [128, S], F32, isOutput=False)
    rot = nc.declare_dram_parameter("rot", [128, 128], F32R, isOutput=False)
    ident = nc.declare_dram_parameter("ident", [128, 128], F32R, isOutput=False)
    tri = nc.declare_dram_parameter("tri", [128, 128], F32R, isOutput=False)
    ones1 = nc.declare_dram_parameter("ones1", [1, 64], F32R, isOutput=False)
    onescol = nc.declare_dram_parameter("onescol", [128, 1], F32R, isOutput=False)
    out = nc.declare_dram_parameter("out", [CHW, H], F32, isOutput=True)

    with ExitStack() as ctx:
        tc = ctx.enter_context(tile.TileContext(nc))
        ctx.enter_context(nc.allow_low_precision(reason="fp32r matmul pipeline"))

        const = ctx.enter_context(tc.tile_pool(name="const", bufs=1))
        xpool = ctx.enter_context(tc.tile_pool(name="xpool", bufs=2))
        wpool = ctx.enter_context(tc.tile_pool(name="wpool", bufs=1))
        qkv = ctx.enter_context(tc.tile_pool(name="qkv", bufs=1))
        work = ctx.enter_context(tc.tile_pool(name="work", bufs=2))
        probs_pool = ctx.enter_context(tc.tile_pool(name="probs_pool", bufs=3))
        attn_pool = ctx.enter_context(tc.tile_pool(name="attn_pool", bufs=1))
        dram = ctx.enter_context(tc.tile_pool(name="dram", bufs=1, space="DRAM"))

        pp = ctx.enter_context(tc.tile_pool(name="pp", bufs=3, space="PSUM"))
        sp = ctx.enter_context(tc.tile_pool(name="sp", bufs=2, space="PSUM"))
        vp = ctx.enter_context(tc.tile_pool(name="vp", bufs=2, space="PSUM"))
        op = ctx.enter_context(tc.tile_pool(name="op", bufs=1, space="PSUM"))

        # ---- constants & inputs to SBUF ----
        rot_t = const.tile([128, 128], F32R)
        ident_t = const.tile([128, 128], F32R)
        tri_t = const.tile([128, 128], F32R)
        ones1_t = const.tile([1, 64], F32R)
        onescol_t = const.tile([128, 1], F32R)
        ct2_t = const.tile([128, S], F32)
        st2_t = const.tile([128, S], F32)
        nc.sync.dma_start(rot_t[:], rot[:])
        nc.sync.dma_start(ident_t[:], ident[:])
        nc.sync.dma_start(tri_t[:], tri[:])
        nc.sync.dma_start(ones1_t[:], ones1[:])
        nc.sync.dma_start(onescol_t[:], onescol[:])
        nc.sync.dma_start(ct2_t[:], ct2[:])
        nc.sync.dma_start(st2_t[:], st2[:])

        wq_t = [wpool.tile([P, 256], F32R, name=f"wq{k}") for k in range(8)]
        wkv_t = [wpool.tile([P, 128], F32R, name=f"wkv{k}") for k in range(8)]
        for k in range(8):
            nc.sync.dma_start(wq_t[k][:], wq[k * P:(k + 1) * P, :])
            nc.sync.dma_start(wkv_t[k][:], wkv[k * P:(k + 1) * P, :])
        wo_t = [wpool.tile([P, H], F32R, name=f"wo{k}") for k in range(2)]
        for k in range(2):
            nc.sync.dma_start(wo_t[k][:], wo[k * P:(k + 1) * P, :])

        # ---- persistent intermediates ----
        # qT: one [128, S] tile per head pair (rows 0-63 head 2p, 64-127 head 2p+1)
        qT = [qkv.tile([P, S], F32R, name=f"qT{p}") for p in range(2)]
        # kT2: k^T duplicated into both halves (lets odd heads use base=64 APs)
        kT2 = qkv.tile([P, S], F32R)
        # v_aug: per seq tile [128, 65]: cols 0-63 v rows, col 64 ones
        v_aug = [qkv.tile([P, 65], F32R, name=f"vaug{t}") for t in range(NT)]

        # ---- projections + RoPE, streamed per 512-wide seq chunk ----
        for n in range(NCH):
            cs = slice(n * CHW, (n + 1) * CHW)
            xc = [xpool.tile([P, CHW], F32R, name=f"xc{k}") for k in range(8)]
            for k in range(8):
                nc.sync.dma_start(xc[k][:], xT[k * P:(k + 1) * P, cs])

            # kv projection: k^T + RoPE (duplicated), v via PE transpose
            kvp = pp.tile([P, CHW], F32, name="kvp", tag="pj")
            for kt in range(8):
                nc.tensor.matmul(kvp[:], wkv_t[kt][:], xc[kt][:],
                                 start=(kt == 0), stop=(kt == 7))
            kraw = work.tile([64, CHW], F32R, name="kraw")
            nc.vector.tensor_copy(kraw[:], kvp[0:64, :])
            krp = pp.tile([64, CHW], F32, name="krp", tag="pj")
            nc.tensor.matmul(krp[:], rot_t[0:64, 0:64], kraw[:],
                             start=True, stop=True)
            ktm1 = work.tile([64, CHW], F32, name="ktm1")
            nc.vector.tensor_tensor(ktm1[:], kvp[0:64, :], ct2_t[0:64, cs], MUL)
            nc.vector.tensor_tensor(kT2[0:64, cs], krp[:], st2_t[0:64, cs], MUL)
            nc.vector.tensor_tensor(kT2[0:64, cs], kT2[0:64, cs], ktm1[:], ADD)
            nc.vector.tensor_copy(kT2[64:128, cs], kT2[0:64, cs])
            # v^T rows 64-127 of kvp -> vT sbuf, then transpose per 128-block
            vTs = work.tile([64, CHW], F32R, name="vTs")
            nc.vector.tensor_copy(vTs[:], kvp[64:128, :])
            for j in range(CHW // P):
                t = n * (CHW // P) + j
                tp = pp.tile([P, 64], F32R, name="tp", tag="pj")
                nc.tensor.transpose(tp[:], vTs[:, j * P:(j + 1) * P],
                                    ident_t[0:64, 0:64])
                nc.vector.tensor_copy(v_aug[t][:, 0:64], tp[:])
                nc.vector.tensor_copy(v_aug[t][:, 64:65], onescol_t[:])

            # q projection + RoPE per head pair
            for pr in range(2):
                qp = pp.tile([P, CHW], F32, name="qp", tag="pj")
                for kt in range(8):
                    nc.tensor.matmul(qp[:], wq_t[kt][:, pr * P:(pr + 1) * P],
                                     xc[kt][:],
                                     start=(kt == 0), stop=(kt == 7))
                qraw = work.tile([P, CHW], F32R, name="qraw")
                nc.vector.tensor_copy(qraw[:], qp[:])
                rp = pp.tile([P, CHW], F32, name="rp", tag="pj")
                nc.tensor.matmul(rp[:], rot_t[:], qraw[:], start=True, stop=True)
                tmp1 = work.tile([P, CHW], F32, name="tmp1")
                nc.vector.tensor_tensor(tmp1[:], qp[:], ct2_t[:, cs], MUL)
                nc.vector.tensor_tensor(qT[pr][:, cs], rp[:], st2_t[:, cs], MUL)
                nc.vector.tensor_tensor(qT[pr][:, cs], qT[pr][:, cs],
                                        tmp1[:], ADD)

        # ---- attention + chunked o-proj + grouped ReduceScatter ----
        rg = [[0, 1, 2, 3], [4, 5, 6, 7]]
        for c in range(NCH):
            base = c * CHW
            at_c = [attn_pool.tile([P, CHW], F32R, name=f"at{c}_{kt}")
                    for kt in range(2)]
            for h in range(4):
                pr, off = h // 2, (h % 2) * 64
                pv = vp.tile([65, CHW], F32, name="pv")
                nsk = 4 * c + 4
                for sk in range(nsk):
                    j = sk - 4 * c
                    lo = max(0, j * P)
                    N = CHW - lo
                    sc = sp.tile([P, CHW], F32, name="sc", tag="sc")
                    nc.tensor.matmul(
                        sc[:, 0:N],
                        kT2[off:off + 64, sk * P:(sk + 1) * P],
                        qT[pr][off:off + 64, base + lo:base + CHW],
                        start=True, stop=True)
                    pb = probs_pool.tile([P, CHW], F32R, name="pb")
                    nc.scalar.activation(pb[:, 0:N], sc[:, 0:N], EXP, scale=0.125)
                    if j >= 0:
                        nc.gpsimd.tensor_tensor(pb[:, 0:P], pb[:, 0:P],
                                                tri_t[:], MUL)
                    nc.tensor.matmul(pv[:, lo:CHW], v_aug[sk][:], pb[:, 0:N],
                                     start=(sk == 0), stop=(sk == nsk - 1))
                # normalize: recip of ones-row, PE-broadcast, multiply
                rcp = work.tile([1, CHW], F32R, name="rcp")
                nc.vector.reciprocal(rcp[:], pv[64:65, :])
                bc = sp.tile([64, CHW], F32, name="bc", tag="sc")
                nc.tensor.matmul(bc[:], ones1_t[:], rcp[:], start=True, stop=True)
                un = work.tile([64, CHW], F32, name="un")
                nc.vector.tensor_copy(un[:], pv[0:64, :])
                nc.vector.tensor_tensor(at_c[pr][off:off + 64, :], un[:],
                                        bc[:], MUL)

            # o-proj for this chunk: out_part[m] = sum_kt atT[kt].T @ wo[kt]
            part = dram.tile([CHW, H], F32, name=f"part{c}")
            for m in range(CHW // P):
                for nh in range(2):
                    po = op.tile([P, CHW], F32, name="po")
                    for kt in range(2):
                        nc.tensor.matmul(po[:], at_c[kt][:, m * P:(m + 1) * P],
                                         wo_t[kt][:, nh * CHW:(nh + 1) * CHW],
                                         start=(kt == 0), stop=(kt == 1))
                    ob = work.tile([P, CHW], F32, name="ob")
                    nc.any.tensor_copy(ob[:], po[:])
                    nc.sync.dma_start(
                        part[m * P:(m + 1) * P, nh * CHW:(nh + 1) * CHW], ob[:])
            rs = dram.tile([P, H], F32, name=f"rs{c}")
            nc.gpsimd.collective_compute(
                "ReduceScatter", mybir.AluOpType.add,
                ins=[part[:]], outs=[rs[:]], replica_groups=rg)
            nc.sync.dma_start(out[c * P:(c + 1) * P, :], rs[:])

    nc.compile()
    return nc


def _host_inputs(hidden_states, cos, sin, Wq, Wk, Wv, Wo):
    x = np.asarray(hidden_states, np.float32)
    cos = np.asarray(cos, np.float32)
    sin = np.asarray(sin, np.float32)
    Wq = np.asarray(Wq, np.float32)
    Wk = np.asarray(Wk, np.float32)
    Wv = np.asarray(Wv, np.float32)
    Wo = np.asarray(Wo, np.float32)

    ct2 = np.ascontiguousarray(np.tile(cos.T, (2, 1)))       # [128, S]
    st2 = np.ascontiguousarray(np.tile(sin.T, (2, 1)))
    r64 = np.zeros((64, 64), np.float32)
    for i in range(32):
        r64[32 + i, i] = -1.0
        r64[i, 32 + i] = 1.0
    rot = np.zeros((128, 128), np.float32)
    rot[0:64, 0:64] = r64
    rot[64:128, 64:128] = r64
    ident = np.eye(128, dtype=np.float32)
    tri = np.triu(np.ones((128, 128), np.float32))
    ones1 = np.ones((1, 64), np.float32)
    onescol = np.ones((128, 1), np.float32)

    xTs = [np.ascontiguousarray(x[d].T) for d in range(B)]
    in_maps = []
    for c_id in range(NCORES):
        d, g = c_id // 4, c_id % 4
        in_maps.append({
            "xT": xTs[d],
            "wq": np.ascontiguousarray(Wq[:, g * 256:(g + 1) * 256]),
            "wkv": np.ascontiguousarray(
                np.concatenate([Wk[:, g * 64:(g + 1) * 64],
                                Wv[:, g * 64:(g + 1) * 64]], axis=1)),
            "wo": np.ascontiguousarray(Wo[g * 256:(g + 1) * 256, :]),
            "ct2": ct2, "st2": st2, "rot": rot, "ident": ident,
            "tri": tri, "ones1": ones1, "onescol": onescol,
        })
    return in_maps


def _assemble(results):
    full = np.empty((B, S, H), np.float32)
    for c_id in range(NCORES):
        d, g = c_id // 4, c_id % 4
        o = np.asarray(results[c_id]["out"])
        for c in range(NCH):
            r0 = c * CHW + g * P
            full[d, r0:r0 + P, :] = o[c * P:(c + 1) * P, :]
    return full


def kernel(hidden_states, cos, sin, attention_mask, Wq, Wk, Wv, Wo):
    from concourse.bass_utils import run_bass_kernel_spmd
    if "nc" not in _prog_cache:
        _prog_cache["nc"] = _build()
    nc = _prog_cache["nc"]
    in_maps = _host_inputs(hidden_states, cos, sin, Wq, Wk, Wv, Wo)
    res = run_bass_kernel_spmd(nc, in_maps, list(range(NCORES)))
    return _assemble(res.results)



# revision 6
# speedup vs baseline: 1.2711x; 1.2711x over previous
"""MiniMind GQA attention block on 8 trn2 NeuronCores.

Sharding (per the TP-by-head hint): core c = (d, g) with d = c // 4 the
batch index (data parallel) and g = c % 4 the KV group (tensor parallel
over heads).  Each core computes q/k/v projections for its 4 query heads
and 1 KV head, RoPE, causal attention, and a partial output projection
through its slice of Wo rows; a grouped bf16 ReduceScatter (groups
[0-3], [4-7]) sums the partials straight into the output parameter,
leaving each core a distinct 128-row shard per 512-row sequence chunk.
The host only slices/casts inputs and concatenates output shards.

All matmul operands are bf16 (fp32 PSUM accumulate); everything on-chip
runs transposed (feature dims on partitions) so the softmax denominator
folds into the PV matmul via a v|ones stationary operand and no
probability transpose is ever needed.
"""

import numpy as np
from contextlib import ExitStack

B, S, H = 2, 2048, 1024
NH, NKV, HD = 16, 4, 64
P = 128
NCH = 4                # 512-wide sequence chunks
CHW = S // NCH         # 512
NCORES = 8

_prog_cache = {}


def _build():
    import concourse.bacc as bacc
    import concourse.mybir as mybir
    from concourse import tile

    F32 = mybir.dt.float32
    F32R = mybir.dt.float32r
    BF16 = mybir.dt.bfloat16
    EXP = mybir.ActivationFunctionType.Exp
    CPY = mybir.ActivationFunctionType.Copy
    MUL = mybir.AluOpType.mult
    ADD = mybir.AluOpType.add

    nc = bacc.Bacc()

    xT = nc.declare_dram_parameter("xT", [H, S], BF16, isOutput=False)
    wq = nc.declare_dram_parameter("wq", [H, 256], BF16, isOutput=False)
    wkv = nc.declare_dram_parameter("wkv", [H, 128], BF16, isOutput=False)
    wo = nc.declare_dram_parameter("wo", [256, H], BF16, isOutput=False)
    ct2 = nc.declare_dram_parameter("ct2", [128, S], F32, isOutput=False)
    st2 = nc.declare_dram_parameter("st2", [128, S], F32, isOutput=False)
    rot = nc.declare_dram_parameter("rot", [128, 128], BF16, isOutput=False)
    ident = nc.declare_dram_parameter("ident", [64, 64], BF16, isOutput=False)
    tri = nc.declare_dram_parameter("tri", [128, 128], BF16, isOutput=False)
    ones1 = nc.declare_dram_parameter("ones1", [1, 64], F32R, isOutput=False)
    onescol = nc.declare_dram_parameter("onescol", [128, 1], BF16,
                                        isOutput=False)
    out = nc.declare_dram_parameter("out", [CHW, H], BF16, isOutput=True)

    with ExitStack() as ctx:
        tc = ctx.enter_context(tile.TileContext(nc))
        ctx.enter_context(nc.allow_low_precision(reason="bf16 pipeline"))

        const = ctx.enter_context(tc.tile_pool(name="const", bufs=1))
        xpool = ctx.enter_context(tc.tile_pool(name="xpool", bufs=2))
        wpool = ctx.enter_context(tc.tile_pool(name="wpool", bufs=1))
        qkv = ctx.enter_context(tc.tile_pool(name="qkv", bufs=1))
        work = ctx.enter_context(tc.tile_pool(name="work", bufs=3))
        probs_pool = ctx.enter_context(tc.tile_pool(name="probs_pool", bufs=3))
        attn_pool = ctx.enter_context(tc.tile_pool(name="attn_pool", bufs=2))
        obuf = ctx.enter_context(tc.tile_pool(name="obuf", bufs=3))
        dram = ctx.enter_context(tc.tile_pool(name="dram", bufs=2,
                                              space="DRAM"))

        # PSUM: pp 2 banks + sp 2x[128,1024]=4 banks + vp 2 banks = 8
        pp = ctx.enter_context(tc.tile_pool(name="pp", bufs=2, space="PSUM"))
        sp = ctx.enter_context(tc.tile_pool(name="sp", bufs=2, space="PSUM"))
        vp = ctx.enter_context(tc.tile_pool(name="vp", bufs=2, space="PSUM"))

        # ---- constants & weights to SBUF ----
        rot_t = const.tile([128, 128], BF16)
        ident_t = const.tile([64, 64], BF16)
        tri_t = const.tile([128, 128], BF16)
        ones1_t = const.tile([1, 64], F32R)
        onescol_t = const.tile([128, 1], BF16)
        ct2_t = const.tile([128, S], F32)
        st2_t = const.tile([128, S], F32)
        nc.sync.dma_start(rot_t[:], rot[:])
        nc.sync.dma_start(ident_t[:], ident[:])
        nc.sync.dma_start(tri_t[:], tri[:])
        nc.sync.dma_start(ones1_t[:], ones1[:])
        nc.sync.dma_start(onescol_t[:], onescol[:])

        wkv_t = [wpool.tile([P, 128], BF16, name=f"wkv{k}") for k in range(8)]
        for k in range(8):
            nc.sync.dma_start(wkv_t[k][:], wkv[k * P:(k + 1) * P, :])
        nc.sync.dma_start(ct2_t[:], ct2[:])
        nc.sync.dma_start(st2_t[:], st2[:])
        wq_t = [wpool.tile([P, 256], BF16, name=f"wq{k}") for k in range(8)]
        for k in range(8):
            nc.sync.dma_start(wq_t[k][:], wq[k * P:(k + 1) * P, :])
        wo_t = [wpool.tile([P, H], BF16, name=f"wo{k}") for k in range(2)]

        # ---- persistent intermediates ----
        # qT: one [128, S] tile per head pair (rows 0-63 head 2p, 64-127
        # head 2p+1); kT2: k^T duplicated into both halves (odd heads use
        # base=64 APs); v_aug per seq tile: cols 0-63 v rows, col 64 ones.
        qT = [qkv.tile([P, S], BF16, name=f"qT{p}") for p in range(2)]
        kT2 = qkv.tile([P, S], BF16)
        v_aug = [qkv.tile([P, 66], BF16, name=f"vaug{t}")
                 for t in range(S // P)]

        rg = [[0, 1, 2, 3], [4, 5, 6, 7]]

        for n in range(NCH):
            cs = slice(n * CHW, (n + 1) * CHW)
            base = n * CHW

            # ---- projections + RoPE for this chunk ----
            xc = [xpool.tile([P, CHW], BF16, name=f"xc{k}") for k in range(8)]
            for k in range(8):
                nc.sync.dma_start(xc[k][:], xT[k * P:(k + 1) * P, cs])

            # kv projection: k^T + RoPE (duplicated), v via PE transpose
            kvp = pp.tile([P, CHW], F32, name="kvp", tag="pj")
            for kt in range(8):
                nc.tensor.matmul(kvp[:], wkv_t[kt][:], xc[kt][:],
                                 start=(kt == 0), stop=(kt == 7))
            kraw = work.tile([64, CHW], BF16, name="kraw")
            nc.vector.tensor_copy(kraw[:], kvp[0:64, :])
            krp = pp.tile([64, CHW], F32, name="krp", tag="pj")
            nc.tensor.matmul(krp[:], rot_t[0:64, 0:64], kraw[:],
                             start=True, stop=True)
            ktm1 = work.tile([64, CHW], F32, name="ktm1")
            nc.vector.tensor_tensor(ktm1[:], kvp[0:64, :], ct2_t[0:64, cs],
                                    MUL)
            ktm2 = work.tile([64, CHW], F32, name="ktm2")
            nc.vector.tensor_tensor(ktm2[:], krp[:], st2_t[0:64, cs], MUL)
            nc.vector.tensor_tensor(kT2[0:64, cs], ktm2[:], ktm1[:], ADD)
            nc.vector.tensor_copy(kT2[64:128, cs], kT2[0:64, cs])
            # v^T rows 64-127 of kvp -> vT sbuf, transpose per 128-block
            vTs = work.tile([64, CHW], BF16, name="vTs")
            nc.vector.tensor_copy(vTs[:], kvp[64:128, :])
            for j in range(CHW // P):
                t = n * (CHW // P) + j
                tp = pp.tile([P, 64], BF16, name="tp", tag="pj")
                nc.tensor.transpose(tp[:], vTs[:, j * P:(j + 1) * P],
                                    ident_t[:])
                nc.vector.tensor_copy(v_aug[t][:, 0:64], tp[:])
                nc.vector.tensor_copy(v_aug[t][:, 64:65], onescol_t[:])

            # q projection + RoPE per head pair
            for pr in range(2):
                qp = pp.tile([P, CHW], F32, name="qp", tag="pj")
                for kt in range(8):
                    nc.tensor.matmul(qp[:], wq_t[kt][:, pr * P:(pr + 1) * P],
                                     xc[kt][:],
                                     start=(kt == 0), stop=(kt == 7))
                qraw = work.tile([P, CHW], BF16, name="qraw")
                nc.vector.tensor_copy(qraw[:], qp[:])
                rp = pp.tile([P, CHW], F32, name="rp", tag="pj")
                nc.tensor.matmul(rp[:], rot_t[:], qraw[:], start=True,
                                 stop=True)
                tmp1 = work.tile([P, CHW], F32, name="tmp1")
                nc.vector.tensor_tensor(tmp1[:], qp[:], ct2_t[:, cs], MUL)
                tmp2 = work.tile([P, CHW], F32, name="tmp2")
                nc.vector.tensor_tensor(tmp2[:], rp[:], st2_t[:, cs], MUL)
                nc.vector.tensor_tensor(qT[pr][:, cs], tmp2[:], tmp1[:], ADD)

            if n == 0:
                for k in range(2):
                    nc.sync.dma_start(wo_t[k][:], wo[k * P:(k + 1) * P, :])

            # ---- attention for this chunk (4 heads) ----
            at_c = [attn_pool.tile([P, CHW], BF16, name=f"at{n}_{kt}")
                    for kt in range(2)]
            for h in range(4):
                pr, off = h // 2, (h % 2) * 64
                pv = vp.tile([65, CHW], F32, name="pv", tag="pv")
                nsk = 4 * n + 4
                first = True
                # full key tiles, two per PSUM/exp group
                for fg in range(n * 2):
                    sk0 = fg * 2
                    sc = sp.tile([P, 2 * CHW], F32, name="sc", tag="sc")
                    for u in range(2):
                        sk = sk0 + u
                        nc.tensor.matmul(
                            sc[:, u * CHW:(u + 1) * CHW],
                            kT2[off:off + 64, sk * P:(sk + 1) * P],
                            qT[pr][off:off + 64, cs],
                            start=True, stop=True)
                    pb = probs_pool.tile([P, 2 * CHW], BF16, name="pb")
                    nc.scalar.activation(pb[:], sc[:], EXP, scale=0.125)
                    for u in range(2):
                        sk = sk0 + u
                        nc.tensor.matmul(pv[:], v_aug[sk][:, 0:65],
                                         pb[:, u * CHW:(u + 1) * CHW],
                                         start=first, stop=False)
                        first = False
                # diagonal key tiles: (j=0, j=1) then (j=2, j=3)
                for dg in range(2):
                    js = (0, 1) if dg == 0 else (2, 3)
                    ws = [CHW - j * P for j in js]
                    sc = sp.tile([P, 2 * CHW], F32, name="scd", tag="sc")
                    cols = []
                    o = 0
                    for j, w in zip(js, ws):
                        sk = 4 * n + j
                        nc.tensor.matmul(
                            sc[:, o:o + w],
                            kT2[off:off + 64, sk * P:(sk + 1) * P],
                            qT[pr][off:off + 64, base + j * P:base + CHW],
                            start=True, stop=True)
                        cols.append(o)
                        o += w
                    pb = probs_pool.tile([P, 2 * CHW], BF16, name="pbd")
                    nc.scalar.activation(pb[:, 0:o], sc[:, 0:o], EXP,
                                         scale=0.125)
                    for j, w, o0 in zip(js, ws, cols):
                        sk = 4 * n + j
                        nc.gpsimd.tensor_tensor(pb[:, o0:o0 + P],
                                                pb[:, o0:o0 + P],
                                                tri_t[:], MUL)
                        last = (dg == 1 and j == js[-1])
                        nc.tensor.matmul(pv[:, j * P:CHW],
                                         v_aug[sk][:, 0:65],
                                         pb[:, o0:o0 + w],
                                         start=first, stop=last)
                        first = False
                # normalize: approx-recip of ones-row, PE-broadcast, mult
                rcp_r = work.tile([1, CHW], F32R, name="rcp_r")
                nc.vector.reciprocal(rcp_r[:], pv[64:65, :])
                bc = vp.tile([64, CHW], F32, name="bc", tag="pv")
                nc.tensor.matmul(bc[:], ones1_t[:],
                                 rcp_r[:], start=True, stop=True)
                un = work.tile([64, CHW], F32, name="un")
                nc.scalar.activation(un[:], pv[0:64, :], CPY)
                nc.vector.tensor_tensor(at_c[pr][off:off + 64, :], un[:],
                                        bc[:], MUL)

            # ---- partial o-proj for this chunk + grouped ReduceScatter ----
            part = dram.tile([CHW, H], BF16, name=f"part{n}")
            for m in range(4):
                for nh in range(2):
                    po = pp.tile([P, CHW], F32, name="po", tag="pj")
                    for kt in range(2):
                        nc.tensor.matmul(po[:], at_c[kt][:, m * P:(m + 1) * P],
                                         wo_t[kt][:, nh * CHW:(nh + 1) * CHW],
                                         start=(kt == 0), stop=(kt == 1))
                    ob = obuf.tile([P, CHW], BF16, name="ob")
                    if nh == 0:
                        nc.vector.tensor_copy(ob[:], po[:])
                    else:
                        nc.scalar.activation(ob[:], po[:], CPY)
                    nc.sync.dma_start(
                        part[m * P:(m + 1) * P, nh * CHW:(nh + 1) * CHW],
                        ob[:])
            rs = dram.tile([P, H], BF16, name=f"rs{n}")
            nc.gpsimd.collective_compute(
                "ReduceScatter", mybir.AluOpType.add,
                ins=[part[:]], outs=[rs[:]], replica_groups=rg)
            nc.sync.dma_start(out[n * P:(n + 1) * P, :], rs[:])

    nc.compile()
    return nc


def _host_inputs(hidden_states, cos, sin, Wq, Wk, Wv, Wo):
    import ml_dtypes

    bf16 = ml_dtypes.bfloat16
    x = np.asarray(hidden_states, np.float32)
    cos = np.asarray(cos, np.float32)
    sin = np.asarray(sin, np.float32)
    Wq = np.asarray(Wq, np.float32)
    Wk = np.asarray(Wk, np.float32)
    Wv = np.asarray(Wv, np.float32)
    Wo = np.asarray(Wo, np.float32)

    ct2 = np.ascontiguousarray(np.tile(cos.T, (2, 1)))       # [128, S]
    st2 = np.ascontiguousarray(np.tile(sin.T, (2, 1)))
    r64 = np.zeros((64, 64), np.float32)
    for i in range(32):
        r64[32 + i, i] = -1.0
        r64[i, 32 + i] = 1.0
    rot = np.zeros((128, 128), np.float32)
    rot[0:64, 0:64] = r64
    rot[64:128, 64:128] = r64
    ident = np.eye(64, dtype=np.float32)
    tri = np.triu(np.ones((128, 128), np.float32))
    ones1 = np.ones((1, 64), np.float32)
    onescol = np.ones((128, 1), np.float32)

    xTs = [np.ascontiguousarray(x[d].T.astype(bf16)) for d in range(B)]
    in_maps = []
    for c_id in range(NCORES):
        d, g = c_id // 4, c_id % 4
        in_maps.append({
            "xT": xTs[d],
            "wq": np.ascontiguousarray(
                Wq[:, g * 256:(g + 1) * 256].astype(bf16)),
            "wkv": np.ascontiguousarray(
                np.concatenate([Wk[:, g * 64:(g + 1) * 64],
                                Wv[:, g * 64:(g + 1) * 64]],
                               axis=1).astype(bf16)),
            "wo": np.ascontiguousarray(
                Wo[g * 256:(g + 1) * 256, :].astype(bf16)),
            "ct2": ct2, "st2": st2,
            "rot": rot.astype(bf16), "ident": ident.astype(bf16),
            "tri": tri.astype(bf16), "ones1": ones1,
            "onescol": onescol.astype(bf16),
        })
    return in_maps


def _assemble(results):
    full = np.empty((B, S, H), np.float32)
    for c_id in range(NCORES):
        d, g = c_id // 4, c_id % 4
        o = np.asarray(results[c_id]["out"]).astype(np.float32)
        for c in range(NCH):
            r0 = c * CHW + g * P
            full[d, r0:r0 + P, :] = o[c * P:(c + 1) * P, :]
    return full


def kernel(hidden_states, cos, sin, attention_mask, Wq, Wk, Wv, Wo):
    from concourse.bass_utils import run_bass_kernel_spmd
    if "nc" not in _prog_cache:
        _prog_cache["nc"] = _build()
    nc = _prog_cache["nc"]
    in_maps = _host_inputs(hidden_states, cos, sin, Wq, Wk, Wv, Wo)
    res = run_bass_kernel_spmd(nc, in_maps, list(range(NCORES)))
    return _assemble(res.results)
